# revision 12
# baseline (speedup 1.0000x reference)
"""Trainium2 Bass kernel for nn_MultiHeadAttention_7739531067803.

Dual-stream (node/pos) multi-head attention with two shared softmaxes.
Sharding: data-parallel over batch (16 batches -> 2 per core x 8 cores),
zero collectives.

Per-core dataflow (all matmuls bf16 with fp32 PSUM accumulation):
  X [tok,d] --PE transpose--> X^T [d,tok]
  Q^T/K^T [hk,tok] = W^T @ X^T   (head-pair packed: 2 heads x 64 per tile)
  V [tok, (i,h,src,k)] = X^T.T @ Wv  (node/pos interleaved per head-group)
  S^T [n,q] = K^T.T @ Q^T per (head, attn)   (K=64 contraction)
  P^T = exp(S^T/8) via ACT (no max subtraction; logits bounded ~24)
  O^T [vcat,q] = V.T @ P^T ; sums[q] = ones.T @ P^T  (partition-dim softmax sum)
  recip = 1/broadcast(sums) via ones-matmul; heads^T = O^T * recip
  out[q,e] = heads.T @ Wout accumulated over heads
"""

import math

import numpy as np
import ml_dtypes

import concourse.bacc as bacc
import concourse.mybir as mybir
import concourse.tile as tile
from concourse.bass_utils import run_bass_kernel_spmd
from concourse.masks import make_identity

dt = mybir.dt
BF = dt.bfloat16
F32 = dt.float32
F32R = dt.float32r
EXP = mybir.ActivationFunctionType.Exp

N_CORES = 8
B_LOC = 2          # batches per core
NTOK = 1024        # tokens (graph nodes)
D = 512            # input dim
H = 8              # heads
KD = 64            # head dim
E = 512            # embed dim
NT = NTOK // 128   # 8 token tiles
DT = D // 128      # 4 contraction tiles
QH = 2             # 512-wide query halves
NORM = 1.0 / math.sqrt(KD)

_CACHE = {}


def _build():
    nc = bacc.Bacc("TRN2", debug=False, enable_asserts=False)

    x_nd = nc.dram_tensor("x_node", [B_LOC, NTOK, D], BF, kind="ExternalInput").ap()
    x_pd = nc.dram_tensor("x_pos", [B_LOC, NTOK, D], BF, kind="ExternalInput").ap()
    wqk_d = [
        nc.dram_tensor(nm, [D, H * KD], BF, kind="ExternalInput").ap()
        for nm in ("w_qn", "w_qp", "w_kn", "w_kp")
    ]
    wv_d = [
        nc.dram_tensor(nm, [D, 2 * H * KD], BF, kind="ExternalInput").ap()
        for nm in ("w_vn", "w_vp")
    ]
    wo_d = [
        nc.dram_tensor(nm, [H, 2 * KD, E], BF, kind="ExternalInput").ap()
        for nm in ("w_on", "w_op")
    ]
    out_d = [
        nc.dram_tensor(nm, [B_LOC, NTOK, E], F32, kind="ExternalOutput").ap()
        for nm in ("out_node", "out_pos")
    ]

    with tile.TileContext(nc) as tc:
        with (
            tc.tile_pool(name="const", bufs=1) as constp,
            tc.tile_pool(name="wsb", bufs=1) as wp,
            tc.tile_pool(name="xraw", bufs=8) as xrawp,
            tc.tile_pool(name="xt", bufs=8) as xtp,
            tc.tile_pool(name="vsb", bufs=9) as vp,
            tc.tile_pool(name="qkt", bufs=19) as qktp,
            tc.tile_pool(name="heads", bufs=16) as headsp,
            tc.tile_pool(name="pt", bufs=6) as ptp,
            tc.tile_pool(name="recip", bufs=2) as recipp,
            tc.tile_pool(name="ssb", bufs=2) as ssbp,
            tc.tile_pool(name="xs", bufs=3) as xsp,
            tc.tile_pool(name="ostage", bufs=3) as ostagep,
            tc.tile_pool(name="bank", bufs=4, space="PSUM") as pbank,
            tc.tile_pool(name="stp", bufs=4, space="PSUM") as stp,
        ):
            ident = constp.tile([128, 128], BF, tag="ident")
            make_identity(nc, ident)
            ones_col = constp.tile([128, 1], BF, tag="ones_col")
            nc.vector.memset(ones_col, 1.0)

            # prefetch first batch's X tiles before weights (startup critical path)
            xr_pre = {}
            for src, xd in ((0, x_nd), (1, x_pd)):
                for t in range(NT):
                    xr = xrawp.tile([128, D], BF, tag="xraw", name="xrpre")
                    nc.sync.dma_start(out=xr, in_=xd[0, t * 128:(t + 1) * 128, :])
                    xr_pre[(src, t)] = xr

            # persistent weights
            wqk = [[wp.tile([128, 512], BF, tag=f"wqk{t}_{c}", name=f"wqk{t}_{c}") for c in range(DT)]
                   for t in range(4)]
            for t in range(4):
                for c in range(DT):
                    nc.sync.dma_start(out=wqk[t][c], in_=wqk_d[t][c * 128:(c + 1) * 128, :])
            wv = [[wp.tile([128, 1024], BF, tag=f"wv{s}_{c}", name=f"wv{s}_{c}") for c in range(DT)]
                  for s in range(2)]
            for s in range(2):
                for c in range(DT):
                    nc.sync.dma_start(out=wv[s][c], in_=wv_d[s][c * 128:(c + 1) * 128, :])
            wo = [[wp.tile([128, 512], BF, tag=f"wo{t}_{h}", name=f"wo{t}_{h}") for h in range(H)]
                  for t in range(2)]
            for t in range(2):
                for h in range(H):
                    nc.sync.dma_start(out=wo[t][h], in_=wo_d[t][h, :, :])

            for b in range(B_LOC):
                # ---- Phase A: load X, transpose to X^T ----
                xt = [[xtp.tile([128, NTOK], BF, tag="xt", name="xt") for _ in range(DT)]
                      for _ in range(2)]
                for src, xd in ((0, x_nd), (1, x_pd)):
                    for t in range(NT):
                        if b == 0:
                            xr = xr_pre[(src, t)]
                        else:
                            xr = xrawp.tile([128, D], BF, tag="xraw")
                            nc.sync.dma_start(out=xr, in_=xd[b, t * 128:(t + 1) * 128, :])
                        for c in range(DT):
                            tp = pbank.tile([128, 128], BF, tag="bank")
                            nc.tensor.transpose(tp, xr[:, c * 128:(c + 1) * 128], ident)
                            nc.vector.tensor_copy(xt[src][c][:, t * 128:(t + 1) * 128], tp)

                # ---- Phase A-V: V projection, node/pos interleaved ----
                v_sb = [vp.tile([128, 2048], BF, tag="v", name="v") for _ in range(NT)]
                for nt in range(NT):
                    v4 = v_sb[nt].rearrange("p (g s k) -> p g s k", g=16, s=2)
                    for src in range(2):
                        for ch in range(2):
                            ps = pbank.tile([128, 512], F32, tag="bank")
                            for c in range(DT):
                                nc.tensor.matmul(
                                    ps,
                                    lhsT=xt[src][c][:, nt * 128:(nt + 1) * 128],
                                    rhs=wv[src][c][:, ch * 512:(ch + 1) * 512],
                                    start=c == 0, stop=c == DT - 1,
                                )
                            nc.vector.tensor_copy(
                                v4[:, 8 * ch:8 * ch + 8, src, :],
                                ps.rearrange("p (g k) -> p g k", g=8),
                            )

                # ---- Phase A-QK: Q^T/K^T projections, head-pair packed ----
                # qkt[t][pair]: rows = 2 heads x 64k, cols = tokens
                src_of = (0, 1, 0, 1)  # qn, qp, kn, kp
                qkt = [[qktp.tile([128, NTOK], BF, tag="qkt", name="qkt") for _ in range(4)]
                       for _ in range(4)]
                for t in range(4):
                    for pr in range(4):
                        for qh in range(QH):
                            ps = pbank.tile([128, 512], F32, tag="bank")
                            for c in range(DT):
                                nc.tensor.matmul(
                                    ps,
                                    lhsT=wqk[t][c][:, pr * 128:(pr + 1) * 128],
                                    rhs=xt[src_of[t]][c][:, qh * 512:(qh + 1) * 512],
                                    start=c == 0, stop=c == DT - 1,
                                )
                            nc.vector.tensor_copy(
                                qkt[t][pr][:, qh * 512:(qh + 1) * 512], ps)

                # ---- Phase B: attention per (head, attn-stream) ----
                heads = [[headsp.tile([128, NTOK], BF, tag="heads", name="heads") for _ in range(H)]
                         for _ in range(2)]
                for h in range(H):
                    pr, j = h // 2, h % 2
                    prow = slice(j * 64, (j + 1) * 64)
                    for i in range(2):
                        qt_t = qkt[0 if i == 0 else 1][pr]
                        kt_t = qkt[2 if i == 0 else 3][pr]
                        g = i * H + h
                        ots, sumss = [], []
                        for qh in range(QH):
                            ots.append(pbank.tile([128, 512], F32, tag="bank", name="ot"))
                            sumss.append(pbank.tile([1, 512], F32, tag="bank", name="sums"))
                        for nt in range(NT):
                            for qh in range(QH):
                                st = stp.tile([128, 512], F32, tag="st")
                                pt = ptp.tile([128, 512], BF, tag="pt")
                                nc.tensor.matmul(
                                    st,
                                    lhsT=kt_t[prow, nt * 128:(nt + 1) * 128],
                                    rhs=qt_t[prow, qh * 512:(qh + 1) * 512],
                                    start=True, stop=True,
                                )
                                nc.scalar.activation(pt, st, EXP, scale=NORM)
                                nc.tensor.matmul(
                                    ots[qh],
                                    lhsT=v_sb[nt][:, g * 128:(g + 1) * 128],
                                    rhs=pt,
                                    start=nt == 0, stop=nt == NT - 1,
                                )
                                nc.tensor.matmul(
                                    sumss[qh], lhsT=ones_col, rhs=pt,
                                    start=nt == 0, stop=nt == NT - 1,
                                )
                        for qh in range(QH):
                            ot, sums = ots[qh], sumss[qh]
                            rc1 = ssbp.tile([1, 512], F32, tag="ssb")
                            nc.vector.reciprocal(rc1, sums)
                            rc = recipp.tile([128, 512], F32, tag="recip")
                            nc.gpsimd.partition_broadcast(rc, rc1)
                            qcol = slice(qh * 512, (qh + 1) * 512)
                            # quadrants: (node, i=0) and (pos, i=1) keep partitions
                            if i == 0:
                                nc.vector.tensor_mul(
                                    heads[0][h][0:64, qcol], ot[0:64, :], rc[0:64, :])
                                xs = xsp.tile([64, 512], BF, tag="xs")
                                nc.vector.tensor_mul(xs, ot[64:128, :], rc[64:128, :])
                                nc.sync.dma_start(out=heads[1][h][0:64, qcol], in_=xs)
                            else:
                                nc.vector.tensor_mul(
                                    heads[1][h][64:128, qcol], ot[64:128, :], rc[64:128, :])
                                xs = xsp.tile([64, 512], BF, tag="xs")
                                nc.vector.tensor_mul(xs, ot[0:64, :], rc[0:64, :])
                                nc.sync.dma_start(out=heads[0][h][64:128, qcol], in_=xs)

                # ---- Phase C: output projections ----
                for t in range(2):
                    for qt_i in range(NT):
                        ps = pbank.tile([128, 512], F32, tag="bank")
                        for h in range(H):
                            nc.tensor.matmul(
                                ps,
                                lhsT=heads[t][h][:, qt_i * 128:(qt_i + 1) * 128],
                                rhs=wo[t][h],
                                start=h == 0, stop=h == H - 1,
                            )
                        ob = ostagep.tile([128, 512], F32, tag="ostage")
                        nc.vector.tensor_copy(ob, ps)
                        nc.sync.dma_start(
                            out=out_d[t][b, qt_i * 128:(qt_i + 1) * 128, :], in_=ob)

    nc.compile()
    return nc


def _prep_inputs(h_node_in, h_pos_in, W_query_node, W_query_pos, W_key_node,
                 W_key_pos, W_val_node, W_val_pos, W_out_node, W_out_pos):
    bf = ml_dtypes.bfloat16

    def qk(w):  # [H, D, K] -> [D, H*K]
        return np.ascontiguousarray(
            np.transpose(np.asarray(w), (1, 0, 2)).reshape(D, H * KD)).astype(bf)

    # V weights: [2H, D, K] -> [D, (g=i*8+h, k)] where rows 0..7 -> i=0, 8..15 -> i=1
    def vw(w):
        return np.ascontiguousarray(
            np.transpose(np.asarray(w), (1, 0, 2)).reshape(D, 2 * H * KD)).astype(bf)

    shared = {
        "w_qn": qk(W_query_node), "w_qp": qk(W_query_pos),
        "w_kn": qk(W_key_node), "w_kp": qk(W_key_pos),
        "w_vn": vw(W_val_node), "w_vp": vw(W_val_pos),
        "w_on": np.asarray(W_out_node).astype(bf),
        "w_op": np.asarray(W_out_pos).astype(bf),
    }
    xn = np.asarray(h_node_in).astype(bf)
    xp = np.asarray(h_pos_in).astype(bf)
    in_maps = []
    for c in range(N_CORES):
        m = dict(shared)
        m["x_node"] = np.ascontiguousarray(xn[c * B_LOC:(c + 1) * B_LOC])
        m["x_pos"] = np.ascontiguousarray(xp[c * B_LOC:(c + 1) * B_LOC])
        in_maps.append(m)
    return in_maps


def kernel(**inputs):
    if "nc" not in _CACHE:
        _CACHE["nc"] = _build()
    nc = _CACHE["nc"]
    in_maps = _prep_inputs(**inputs)
    res = run_bass_kernel_spmd(nc, in_maps, list(range(N_CORES)))
    out_node = np.concatenate([res.results[c]["out_node"] for c in range(N_CORES)], axis=0)
    out_pos = np.concatenate([res.results[c]["out_pos"] for c in range(N_CORES)], axis=0)
    return (out_node, out_pos)


if __name__ == "__main__":
    z = np.load("/root/problem/inputs.npz")
    outs = kernel(**{k: z[k] for k in z.files})
    print("out shapes:", outs[0].shape, outs[1].shape)


# revision 20
# speedup vs baseline: 1.0939x; 1.0939x over previous
"""Trainium2 Bass kernel for nn_MultiHeadAttention_7739531067803.

Dual-stream (node/pos) multi-head attention with two shared softmaxes.
Sharding: data-parallel over batch (16 batches -> 2 per core x 8 cores),
zero collectives.

Per-core dataflow (all matmuls bf16 with fp32 PSUM accumulation):
  X [tok,d] --PE transpose--> X^T [d,tok]
  Q^T/K^T [hk,tok] = W^T @ X^T   (head-pair packed: 2 heads x 64 per tile)
  V [tok, (i,h,src,k)] = X^T.T @ Wv  (node/pos interleaved per head-group)
  S^T [n,q] = K^T.T @ Q^T per (head, attn)   (K=64 contraction)
  P^T = exp(S^T/8) via ACT (no max subtraction; logits bounded ~24)
  O^T [vcat,q] = V.T @ P^T ; sums[q] = ones.T @ P^T  (partition-dim softmax sum)
  recip = 1/broadcast(sums) via ones-matmul; heads^T = O^T * recip
  out[q,e] = heads.T @ Wout accumulated over heads
"""

import math

import numpy as np
import ml_dtypes

import concourse.bacc as bacc
import concourse.mybir as mybir
import concourse.tile as tile
from concourse.bass_utils import run_bass_kernel_spmd

dt = mybir.dt
BF = dt.bfloat16
F32 = dt.float32
F32R = dt.float32r
EXP = mybir.ActivationFunctionType.Exp

N_CORES = 8
B_LOC = 2          # batches per core
NTOK = 1024        # tokens (graph nodes)
D = 512            # input dim
H = 8              # heads
KD = 64            # head dim
E = 512            # embed dim
NT = NTOK // 128   # 8 token tiles
DT = D // 128      # 4 contraction tiles
QH = 2             # 512-wide query halves
NORM = 1.0 / math.sqrt(KD)

_CACHE = {}


def _build():
    nc = bacc.Bacc("TRN2", debug=False, enable_asserts=False)

    x_nd = nc.dram_tensor("x_node", [B_LOC, NTOK, D], BF, kind="ExternalInput").ap()
    x_pd = nc.dram_tensor("x_pos", [B_LOC, NTOK, D], BF, kind="ExternalInput").ap()
    wqk_d = [
        nc.dram_tensor(nm, [D, H * KD], BF, kind="ExternalInput").ap()
        for nm in ("w_qn", "w_qp", "w_kn", "w_kp")
    ]
    wv_d = [
        nc.dram_tensor(nm, [D, 2 * H * KD], BF, kind="ExternalInput").ap()
        for nm in ("w_vn", "w_vp")
    ]
    wo_d = [
        nc.dram_tensor(nm, [H, 2 * KD, E], BF, kind="ExternalInput").ap()
        for nm in ("w_on", "w_op")
    ]
    out_d = [
        nc.dram_tensor(nm, [B_LOC, NTOK, E], F32, kind="ExternalOutput").ap()
        for nm in ("out_node", "out_pos")
    ]

    with tile.TileContext(nc) as tc:
        with (
            tc.tile_pool(name="const", bufs=1) as constp,
            tc.tile_pool(name="wsb", bufs=1) as wp,
            tc.tile_pool(name="xt", bufs=8) as xtp,
            tc.tile_pool(name="vsb", bufs=9) as vp,
            tc.tile_pool(name="qkt", bufs=19) as qktp,
            tc.tile_pool(name="heads", bufs=16) as headsp,
            tc.tile_pool(name="pt", bufs=6) as ptp,
            tc.tile_pool(name="recip", bufs=3) as recipp,
            tc.tile_pool(name="oc", bufs=4) as ocp,
            tc.tile_pool(name="ssb", bufs=3) as ssbp,
            tc.tile_pool(name="xs", bufs=4) as xsp,
            tc.tile_pool(name="ostage", bufs=3) as ostagep,
            tc.tile_pool(name="bank", bufs=4, space="PSUM") as pbank,
            tc.tile_pool(name="stp", bufs=4, space="PSUM") as stp,
        ):
            ones_col = constp.tile([128, 1], BF, tag="ones_col")
            nc.vector.memset(ones_col, 1.0)

            # prefetch first batch's X^T tiles (DMA transpose), interleaved with
            # the first-needed weights so the first projection starts early
            xt_pre = [[xtp.tile([128, NTOK], BF, tag="xt", name="xtpre") for _ in range(DT)]
                      for _ in range(2)]
            wqk = [[wp.tile([128, 512], BF, tag=f"wqk{t}_{c}", name=f"wqk{t}_{c}") for c in range(DT)]
                   for t in range(4)]

            def xpre(src, c, qh):
                xd = x_nd if src == 0 else x_pd
                nc.sync.dma_start_transpose(
                    xt_pre[src][c][:, qh * 512:(qh + 1) * 512],
                    xd[0, qh * 512:(qh + 1) * 512, c * 128:(c + 1) * 128])

            wv = [[wp.tile([128, 1024], BF, tag=f"wv{sc}_{c}", name=f"wv{sc}_{c}") for c in range(DT)]
                  for sc in range(2)]
            for c in range(DT):
                nc.sync.dma_start(out=wv[0][c], in_=wv_d[0][c * 128:(c + 1) * 128, :])
            for c in range(DT):
                xpre(0, c, 0)
            for c in range(DT):
                nc.sync.dma_start(out=wv[1][c], in_=wv_d[1][c * 128:(c + 1) * 128, :])
            for c in range(DT):
                xpre(1, c, 0)
            for src in range(2):
                for c in range(DT):
                    xpre(src, c, 1)
            for t in range(4):
                for c in range(DT):
                    nc.sync.dma_start(out=wqk[t][c], in_=wqk_d[t][c * 128:(c + 1) * 128, :])
            wo = [[wp.tile([128, 512], BF, tag=f"wo{t}_{h}", name=f"wo{t}_{h}") for h in range(H)]
                  for t in range(2)]
            for t in range(2):
                for h in range(H):
                    nc.sync.dma_start(out=wo[t][h], in_=wo_d[t][h, :, :])

            for b in range(B_LOC):
                # ---- Phase A: X^T via DMA transpose ----
                if b == 0:
                    xt = xt_pre
                else:
                    xt = [[xtp.tile([128, NTOK], BF, tag="xt", name="xt") for _ in range(DT)]
                          for _ in range(2)]
                    for qh in range(QH):
                        for src, xd in ((0, x_nd), (1, x_pd)):
                            for c in range(DT):
                                nc.sync.dma_start_transpose(
                                    xt[src][c][:, qh * 512:(qh + 1) * 512],
                                    xd[b, qh * 512:(qh + 1) * 512, c * 128:(c + 1) * 128])

                # ---- Phase A-V: V projection, node/pos interleaved ----
                v_sb = [vp.tile([128, 2048], BF, tag="v", name="v") for _ in range(NT)]
                for nt in range(NT):
                    v4 = v_sb[nt].rearrange("p (g s k) -> p g s k", g=16, s=2)
                    for src in range(2):
                        for ch in range(2):
                            ps = pbank.tile([128, 512], F32, tag="bank")
                            for c in range(DT):
                                nc.tensor.matmul(
                                    ps,
                                    lhsT=xt[src][c][:, nt * 128:(nt + 1) * 128],
                                    rhs=wv[src][c][:, ch * 512:(ch + 1) * 512],
                                    start=c == 0, stop=c == DT - 1,
                                )
                            nc.vector.tensor_copy(
                                v4[:, 8 * ch:8 * ch + 8, src, :],
                                ps.rearrange("p (g k) -> p g k", g=8),
                            )

                # ---- Phase A-QK: Q^T/K^T projections, head-pair packed ----
                # qkt[t][pair]: rows = 2 heads x 64k, cols = tokens
                src_of = (0, 1, 0, 1)  # qn, qp, kn, kp
                qkt = [[qktp.tile([128, NTOK], BF, tag="qkt", name="qkt") for _ in range(4)]
                       for _ in range(4)]
                for t in range(4):
                    for pr in range(4):
                        for qh in range(QH):
                            ps = pbank.tile([128, 512], F32, tag="bank")
                            for c in range(DT):
                                nc.tensor.matmul(
                                    ps,
                                    lhsT=wqk[t][c][:, pr * 128:(pr + 1) * 128],
                                    rhs=xt[src_of[t]][c][:, qh * 512:(qh + 1) * 512],
                                    start=c == 0, stop=c == DT - 1,
                                )
                            nc.vector.tensor_copy(
                                qkt[t][pr][:, qh * 512:(qh + 1) * 512], ps)

                # ---- Phase B: attention per (head, attn-stream) ----
                heads = [[headsp.tile([128, NTOK], BF, tag="heads", name="heads") for _ in range(H)]
                         for _ in range(2)]
                for h in range(H):
                    pr, j = h // 2, h % 2
                    prow = slice(j * 64, (j + 1) * 64)
                    for i in range(2):
                        qt_t = qkt[0 if i == 0 else 1][pr]
                        kt_t = qkt[2 if i == 0 else 3][pr]
                        g = i * H + h
                        ots, sumss = [], []
                        for qh in range(QH):
                            ots.append(pbank.tile([128, 512], F32, tag="bank", name="ot"))
                            sumss.append(pbank.tile([1, 512], F32, tag="bank", name="sums"))
                        for nt in range(NT):
                            for qh in range(QH):
                                st = stp.tile([128, 512], F32, tag="st")
                                pt = ptp.tile([128, 512], BF, tag="pt")
                                nc.tensor.matmul(
                                    st,
                                    lhsT=kt_t[prow, nt * 128:(nt + 1) * 128],
                                    rhs=qt_t[prow, qh * 512:(qh + 1) * 512],
                                    start=True, stop=True,
                                )
                                nc.scalar.activation(pt, st, EXP, scale=NORM)
                                nc.tensor.matmul(
                                    ots[qh],
                                    lhsT=v_sb[nt][:, g * 128:(g + 1) * 128],
                                    rhs=pt,
                                    start=nt == 0, stop=nt == NT - 1,
                                )
                                nc.tensor.matmul(
                                    sumss[qh], lhsT=ones_col, rhs=pt,
                                    start=nt == 0, stop=nt == NT - 1,
                                )
                        for qh in range(QH):
                            ot, sums = ots[qh], sumss[qh]
                            oc = ocp.tile([128, 512], F32, tag="oc")
                            nc.vector.tensor_copy(oc, ot)  # frees ot bank early
                            rc1 = ssbp.tile([1, 512], F32, tag="ssb")
                            nc.vector.reciprocal(rc1, sums)
                            rc = recipp.tile([128, 512], F32, tag="recip")
                            nc.gpsimd.partition_broadcast(rc, rc1)
                            qcol = slice(qh * 512, (qh + 1) * 512)
                            # quadrants: (node, i=0) and (pos, i=1) keep partitions
                            if i == 0:
                                nc.vector.tensor_mul(
                                    heads[0][h][0:64, qcol], oc[0:64, :], rc[0:64, :])
                                xs = xsp.tile([64, 512], BF, tag="xs")
                                nc.vector.tensor_mul(xs, oc[64:128, :], rc[64:128, :])
                                nc.sync.dma_start(out=heads[1][h][0:64, qcol], in_=xs)
                            else:
                                nc.vector.tensor_mul(
                                    heads[1][h][64:128, qcol], oc[64:128, :], rc[64:128, :])
                                xs = xsp.tile([64, 512], BF, tag="xs")
                                nc.vector.tensor_mul(xs, oc[0:64, :], rc[0:64, :])
                                nc.sync.dma_start(out=heads[0][h][64:128, qcol], in_=xs)

                # ---- Phase C: output projections ----
                for t in range(2):
                    for qt_i in range(NT):
                        ps = pbank.tile([128, 512], F32, tag="bank")
                        for h in range(H):
                            nc.tensor.matmul(
                                ps,
                                lhsT=heads[t][h][:, qt_i * 128:(qt_i + 1) * 128],
                                rhs=wo[t][h],
                                start=h == 0, stop=h == H - 1,
                            )
                        ob = ostagep.tile([128, 512], F32, tag="ostage")
                        nc.vector.tensor_copy(ob, ps)
                        nc.sync.dma_start(
                            out=out_d[t][b, qt_i * 128:(qt_i + 1) * 128, :], in_=ob)

    nc.compile()
    return nc


def _prep_inputs(h_node_in, h_pos_in, W_query_node, W_query_pos, W_key_node,
                 W_key_pos, W_val_node, W_val_pos, W_out_node, W_out_pos):
    bf = ml_dtypes.bfloat16

    def qk(w):  # [H, D, K] -> [D, H*K]
        return np.ascontiguousarray(
            np.transpose(np.asarray(w), (1, 0, 2)).reshape(D, H * KD)).astype(bf)

    # V weights: [2H, D, K] -> [D, (g=i*8+h, k)] where rows 0..7 -> i=0, 8..15 -> i=1
    def vw(w):
        return np.ascontiguousarray(
            np.transpose(np.asarray(w), (1, 0, 2)).reshape(D, 2 * H * KD)).astype(bf)

    shared = {
        "w_qn": qk(W_query_node), "w_qp": qk(W_query_pos),
        "w_kn": qk(W_key_node), "w_kp": qk(W_key_pos),
        "w_vn": vw(W_val_node), "w_vp": vw(W_val_pos),
        "w_on": np.asarray(W_out_node).astype(bf),
        "w_op": np.asarray(W_out_pos).astype(bf),
    }
    xn = np.asarray(h_node_in).astype(bf)
    xp = np.asarray(h_pos_in).astype(bf)
    in_maps = []
    for c in range(N_CORES):
        m = dict(shared)
        m["x_node"] = np.ascontiguousarray(xn[c * B_LOC:(c + 1) * B_LOC])
        m["x_pos"] = np.ascontiguousarray(xp[c * B_LOC:(c + 1) * B_LOC])
        in_maps.append(m)
    return in_maps


def kernel(**inputs):
    if "nc" not in _CACHE:
        _CACHE["nc"] = _build()
    nc = _CACHE["nc"]
    in_maps = _prep_inputs(**inputs)
    res = run_bass_kernel_spmd(nc, in_maps, list(range(N_CORES)))
    out_node = np.concatenate([res.results[c]["out_node"] for c in range(N_CORES)], axis=0)
    out_pos = np.concatenate([res.results[c]["out_pos"] for c in range(N_CORES)], axis=0)
    return (out_node, out_pos)


if __name__ == "__main__":
    z = np.load("/root/problem/inputs.npz")
    outs = kernel(**{k: z[k] for k in z.files})
    print("out shapes:", outs[0].shape, outs[1].shape)


# revision 22
# speedup vs baseline: 1.1092x; 1.0140x over previous
"""Trainium2 Bass kernel for nn_MultiHeadAttention_7739531067803.

Dual-stream (node/pos) multi-head attention with two shared softmaxes.
Sharding: data-parallel over batch (16 batches -> 2 per core x 8 cores),
zero collectives.

Per-core dataflow (all matmuls bf16 with fp32 PSUM accumulation):
  X [tok,d] --PE transpose--> X^T [d,tok]
  Q^T/K^T [hk,tok] = W^T @ X^T   (head-pair packed: 2 heads x 64 per tile)
  V [tok, (i,h,src,k)] = X^T.T @ Wv  (node/pos interleaved per head-group)
  S^T [n,q] = K^T.T @ Q^T per (head, attn)   (K=64 contraction)
  P^T = exp(S^T/8) via ACT (no max subtraction; logits bounded ~24)
  O^T [vcat,q] = V.T @ P^T ; sums[q] = ones.T @ P^T  (partition-dim softmax sum)
  recip = 1/broadcast(sums) via ones-matmul; heads^T = O^T * recip
  out[q,e] = heads.T @ Wout accumulated over heads
"""

import math

import numpy as np
import ml_dtypes

import concourse.bacc as bacc
import concourse.mybir as mybir
import concourse.tile as tile
from concourse.bass_utils import run_bass_kernel_spmd

dt = mybir.dt
BF = dt.bfloat16
F32 = dt.float32
F32R = dt.float32r
EXP = mybir.ActivationFunctionType.Exp

N_CORES = 8
B_LOC = 2          # batches per core
NTOK = 1024        # tokens (graph nodes)
D = 512            # input dim
H = 8              # heads
KD = 64            # head dim
E = 512            # embed dim
NT = NTOK // 128   # 8 token tiles
DT = D // 128      # 4 contraction tiles
QH = 2             # 512-wide query halves
NORM = 1.0 / math.sqrt(KD)

_CACHE = {}


def _build():
    nc = bacc.Bacc("TRN2", debug=False, enable_asserts=False)

    x_nd = nc.dram_tensor("x_node", [B_LOC, NTOK, D], BF, kind="ExternalInput").ap()
    x_pd = nc.dram_tensor("x_pos", [B_LOC, NTOK, D], BF, kind="ExternalInput").ap()
    wqk_d = [
        nc.dram_tensor(nm, [D, H * KD], BF, kind="ExternalInput").ap()
        for nm in ("w_qn", "w_qp", "w_kn", "w_kp")
    ]
    wv_d = [
        nc.dram_tensor(nm, [D, 2 * H * KD], BF, kind="ExternalInput").ap()
        for nm in ("w_vn", "w_vp")
    ]
    wo_d = [
        nc.dram_tensor(nm, [H, 2 * KD, E], BF, kind="ExternalInput").ap()
        for nm in ("w_on", "w_op")
    ]
    out_d = [
        nc.dram_tensor(nm, [B_LOC, NTOK, E], F32, kind="ExternalOutput").ap()
        for nm in ("out_node", "out_pos")
    ]

    with tile.TileContext(nc) as tc:
        with (
            tc.tile_pool(name="const", bufs=1) as constp,
            tc.tile_pool(name="wsb", bufs=1) as wp,
            tc.tile_pool(name="xt", bufs=8) as xtp,
            tc.tile_pool(name="vsb", bufs=9) as vp,
            tc.tile_pool(name="qkt", bufs=19) as qktp,
            tc.tile_pool(name="heads", bufs=16) as headsp,
            tc.tile_pool(name="pt", bufs=6) as ptp,
            tc.tile_pool(name="recip", bufs=3) as recipp,
            tc.tile_pool(name="oc", bufs=4) as ocp,
            tc.tile_pool(name="ssb", bufs=3) as ssbp,
            tc.tile_pool(name="xs", bufs=4) as xsp,
            tc.tile_pool(name="ostage", bufs=3) as ostagep,
            tc.tile_pool(name="bank", bufs=5, space="PSUM") as pbank,
            tc.tile_pool(name="stp", bufs=3, space="PSUM") as stp,
        ):
            ones_col = constp.tile([128, 1], BF, tag="ones_col")
            nc.vector.memset(ones_col, 1.0)

            # prefetch first batch's X^T tiles (DMA transpose), interleaved with
            # the first-needed weights so the first projection starts early
            xt_pre = [[xtp.tile([128, NTOK], BF, tag="xt", name="xtpre") for _ in range(DT)]
                      for _ in range(2)]
            wqk = [[wp.tile([128, 512], BF, tag=f"wqk{t}_{c}", name=f"wqk{t}_{c}") for c in range(DT)]
                   for t in range(4)]

            def xpre(src, c, qh, split=1):
                xd = x_nd if src == 0 else x_pd
                w = 512 // split
                for sub in range(split):
                    lo = qh * 512 + sub * w
                    nc.sync.dma_start_transpose(
                        xt_pre[src][c][:, lo:lo + w],
                        xd[0, lo:lo + w, c * 128:(c + 1) * 128])

            wv = [[wp.tile([128, 1024], BF, tag=f"wv{sc}_{c}", name=f"wv{sc}_{c}") for c in range(DT)]
                  for sc in range(2)]
            for c in range(DT):
                nc.sync.dma_start(out=wv[0][c], in_=wv_d[0][c * 128:(c + 1) * 128, :])
            for c in range(DT):
                xpre(0, c, 0)
            for c in range(DT):
                nc.sync.dma_start(out=wv[1][c], in_=wv_d[1][c * 128:(c + 1) * 128, :])
            for c in range(DT):
                xpre(1, c, 0)
            for src in range(2):
                for c in range(DT):
                    xpre(src, c, 1)
            for t in range(4):
                for c in range(DT):
                    nc.sync.dma_start(out=wqk[t][c], in_=wqk_d[t][c * 128:(c + 1) * 128, :])
            wo = [[wp.tile([128, 512], BF, tag=f"wo{t}_{h}", name=f"wo{t}_{h}") for h in range(H)]
                  for t in range(2)]
            for t in range(2):
                for h in range(H):
                    nc.sync.dma_start(out=wo[t][h], in_=wo_d[t][h, :, :])

            for b in range(B_LOC):
                # ---- Phase A: X^T via DMA transpose ----
                if b == 0:
                    xt = xt_pre
                else:
                    xt = [[xtp.tile([128, NTOK], BF, tag="xt", name="xt") for _ in range(DT)]
                          for _ in range(2)]
                    for qh in range(QH):
                        for src, xd in ((0, x_nd), (1, x_pd)):
                            for c in range(DT):
                                nc.sync.dma_start_transpose(
                                    xt[src][c][:, qh * 512:(qh + 1) * 512],
                                    xd[b, qh * 512:(qh + 1) * 512, c * 128:(c + 1) * 128])

                # ---- Phase A-V: V projection, node/pos interleaved ----
                v_sb = [vp.tile([128, 2048], BF, tag="v", name="v") for _ in range(NT)]
                for nt in range(NT):
                    v4 = v_sb[nt].rearrange("p (g s k) -> p g s k", g=16, s=2)
                    for src in range(2):
                        for ch in range(2):
                            ps = pbank.tile([128, 512], F32, tag="bank")
                            for c in range(DT):
                                nc.tensor.matmul(
                                    ps,
                                    lhsT=xt[src][c][:, nt * 128:(nt + 1) * 128],
                                    rhs=wv[src][c][:, ch * 512:(ch + 1) * 512],
                                    start=c == 0, stop=c == DT - 1,
                                )
                            nc.vector.tensor_copy(
                                v4[:, 8 * ch:8 * ch + 8, src, :],
                                ps.rearrange("p (g k) -> p g k", g=8),
                            )

                # ---- Phase A-QK: Q^T/K^T projections, head-pair packed ----
                # qkt[t][pair]: rows = 2 heads x 64k, cols = tokens
                src_of = (0, 1, 0, 1)  # qn, qp, kn, kp
                qkt = [[qktp.tile([128, NTOK], BF, tag="qkt", name="qkt") for _ in range(4)]
                       for _ in range(4)]
                for t in range(4):
                    for pr in range(4):
                        for qh in range(QH):
                            ps = pbank.tile([128, 512], F32, tag="bank")
                            for c in range(DT):
                                nc.tensor.matmul(
                                    ps,
                                    lhsT=wqk[t][c][:, pr * 128:(pr + 1) * 128],
                                    rhs=xt[src_of[t]][c][:, qh * 512:(qh + 1) * 512],
                                    start=c == 0, stop=c == DT - 1,
                                )
                            nc.vector.tensor_copy(
                                qkt[t][pr][:, qh * 512:(qh + 1) * 512], ps)

                # ---- Phase B: attention per (head, attn-stream) ----
                heads = [[headsp.tile([128, NTOK], BF, tag="heads", name="heads") for _ in range(H)]
                         for _ in range(2)]
                for h in range(H):
                    pr, j = h // 2, h % 2
                    prow = slice(j * 64, (j + 1) * 64)
                    for i in range(2):
                        qt_t = qkt[0 if i == 0 else 1][pr]
                        kt_t = qkt[2 if i == 0 else 3][pr]
                        g = i * H + h
                        ots, sumss = [], []
                        for qh in range(QH):
                            ots.append(pbank.tile([128, 512], F32, tag="bank", name="ot"))
                            sumss.append(pbank.tile([1, 512], F32, tag="bank", name="sums"))
                        for nt in range(NT):
                            for qh in range(QH):
                                st = stp.tile([128, 512], F32, tag="st")
                                pt = ptp.tile([128, 512], BF, tag="pt")
                                nc.tensor.matmul(
                                    st,
                                    lhsT=kt_t[prow, nt * 128:(nt + 1) * 128],
                                    rhs=qt_t[prow, qh * 512:(qh + 1) * 512],
                                    start=True, stop=True,
                                )
                                nc.scalar.activation(pt, st, EXP, scale=NORM)
                                nc.tensor.matmul(
                                    ots[qh],
                                    lhsT=v_sb[nt][:, g * 128:(g + 1) * 128],
                                    rhs=pt,
                                    start=nt == 0, stop=nt == NT - 1,
                                )
                                nc.tensor.matmul(
                                    sumss[qh], lhsT=ones_col, rhs=pt,
                                    start=nt == 0, stop=nt == NT - 1,
                                )
                        for qh in range(QH):
                            ot, sums = ots[qh], sumss[qh]
                            oc = ocp.tile([128, 512], F32, tag="oc")
                            nc.vector.tensor_copy(oc, ot)  # frees ot bank early
                            rc1 = ssbp.tile([1, 512], F32, tag="ssb")
                            nc.vector.reciprocal(rc1, sums)
                            rc = recipp.tile([128, 512], F32, tag="recip")
                            nc.gpsimd.partition_broadcast(rc, rc1)
                            qcol = slice(qh * 512, (qh + 1) * 512)
                            # quadrants: (node, i=0) and (pos, i=1) keep partitions
                            if i == 0:
                                nc.vector.tensor_mul(
                                    heads[0][h][0:64, qcol], oc[0:64, :], rc[0:64, :])
                                xs = xsp.tile([64, 512], BF, tag="xs")
                                nc.vector.tensor_mul(xs, oc[64:128, :], rc[64:128, :])
                                nc.sync.dma_start(out=heads[1][h][0:64, qcol], in_=xs)
                            else:
                                nc.vector.tensor_mul(
                                    heads[1][h][64:128, qcol], oc[64:128, :], rc[64:128, :])
                                xs = xsp.tile([64, 512], BF, tag="xs")
                                nc.vector.tensor_mul(xs, oc[0:64, :], rc[0:64, :])
                                nc.sync.dma_start(out=heads[0][h][64:128, qcol], in_=xs)

                # ---- Phase C: output projections ----
                for t in range(2):
                    for qt_i in range(NT):
                        ps = pbank.tile([128, 512], F32, tag="bank")
                        for h in range(H):
                            nc.tensor.matmul(
                                ps,
                                lhsT=heads[t][h][:, qt_i * 128:(qt_i + 1) * 128],
                                rhs=wo[t][h],
                                start=h == 0, stop=h == H - 1,
                            )
                        ob = ostagep.tile([128, 512], F32, tag="ostage")
                        nc.vector.tensor_copy(ob, ps)
                        nc.sync.dma_start(
                            out=out_d[t][b, qt_i * 128:(qt_i + 1) * 128, :], in_=ob)

    nc.compile()
    return nc


def _prep_inputs(h_node_in, h_pos_in, W_query_node, W_query_pos, W_key_node,
                 W_key_pos, W_val_node, W_val_pos, W_out_node, W_out_pos):
    bf = ml_dtypes.bfloat16

    def qk(w):  # [H, D, K] -> [D, H*K]
        return np.ascontiguousarray(
            np.transpose(np.asarray(w), (1, 0, 2)).reshape(D, H * KD)).astype(bf)

    # V weights: [2H, D, K] -> [D, (g=i*8+h, k)] where rows 0..7 -> i=0, 8..15 -> i=1
    def vw(w):
        return np.ascontiguousarray(
            np.transpose(np.asarray(w), (1, 0, 2)).reshape(D, 2 * H * KD)).astype(bf)

    shared = {
        "w_qn": qk(W_query_node), "w_qp": qk(W_query_pos),
        "w_kn": qk(W_key_node), "w_kp": qk(W_key_pos),
        "w_vn": vw(W_val_node), "w_vp": vw(W_val_pos),
        "w_on": np.asarray(W_out_node).astype(bf),
        "w_op": np.asarray(W_out_pos).astype(bf),
    }
    xn = np.asarray(h_node_in).astype(bf)
    xp = np.asarray(h_pos_in).astype(bf)
    in_maps = []
    for c in range(N_CORES):
        m = dict(shared)
        m["x_node"] = np.ascontiguousarray(xn[c * B_LOC:(c + 1) * B_LOC])
        m["x_pos"] = np.ascontiguousarray(xp[c * B_LOC:(c + 1) * B_LOC])
        in_maps.append(m)
    return in_maps


def kernel(**inputs):
    if "nc" not in _CACHE:
        _CACHE["nc"] = _build()
    nc = _CACHE["nc"]
    in_maps = _prep_inputs(**inputs)
    res = run_bass_kernel_spmd(nc, in_maps, list(range(N_CORES)))
    out_node = np.concatenate([res.results[c]["out_node"] for c in range(N_CORES)], axis=0)
    out_pos = np.concatenate([res.results[c]["out_pos"] for c in range(N_CORES)], axis=0)
    return (out_node, out_pos)


if __name__ == "__main__":
    z = np.load("/root/problem/inputs.npz")
    outs = kernel(**{k: z[k] for k in z.files})
    print("out shapes:", outs[0].shape, outs[1].shape)
